# revision 25
# baseline (speedup 1.0000x reference)
"""Trainium2 Bass kernel for BasicAttentionModule (pooled attention + residual).

Computation (per sample): 8x8 avg-pool -> 1x1-conv q/k/v over 1024 tokens ->
softmax attention -> nearest 8x upsample -> residual add.

Sharding: 2 cores per sample (batch 4 x 8 cores); each core owns 128 of the
256 rows.  Per core:
  phase 1  stream 16 slabs of [128ch, 16rows, 256] fp32 on the sync HWDGE
           ring (16 KiB packets, ~330-350 GB/s); every slab is cast to a
           persistent bf16 SBUF cache (16 MiB -> NO second feature read) on
           the ACT engine while an all-bf16 DVE tree + reduce pools it into
           bf16 token sums (the tree reads the cache, so the slab ring is
           freed by the cast alone).
  cc       pooled tokens are exchanged pairwise in TWO AllGathers (tokens
           0:256 and 256:512).  Measured constraint: an active collective
           starves every other DMA ring for its whole window (including the
           wait for the slowest core), so cc#1's stores ride the sync DMA
           ring at a FIFO position that opens the window right as the last
           loads complete; cc#2 goes right after.  A PE warm-up burst gated
           on the last slab's cast trips the clock ramp for the attention.
  attn     tokens-on-partitions (no transposes): per 128-token m-chunk,
           v^T and E = exp(energy^T) feed PSUM-accumulated denominator
           (all-ones matmul) and output matmuls; e/vt tiles are fp8
           (energies here are tiny so exp ~= 1; quantization is ~3e-5 of
           the output).  v-bias is folded in AFTER normalization (softmax
           weights sum to 1), so no ones/vb matmuls.  The reciprocal +
           normalize run in column quarters so early output slabs unblock
           first.
  tail     out slab = bf16 cache + upsampled attention (stride-0 broadcast
           AP) on DVE into the recycled fp32 slab ring, stores alternate
           between the scalar and sync DMA rings (~352 GB/s).
"""

import ml_dtypes
import numpy as np

import concourse.bass as bass
import concourse.mybir as mybir
import concourse.tile as tile
from concourse.bass_utils import run_bass_kernel_spmd

F32 = mybir.dt.float32
BF16 = mybir.dt.bfloat16
F8 = mybir.dt.float8e4

B, C, H, W = 4, 256, 256, 256
S = 8                      # pool stride
KCH = 32                   # key channels
N_CORES = 8
HH = H // 2                # 128 rows per core
HP, WP = HH // S, W // S   # 16 x 32 pooled grid per core half
NT = HP * WP               # 512 tokens owned per core
NTOK = 2 * NT              # 1024 tokens per sample
CCH = C // 128             # 2 channel chunks
NJ = NTOK // 128           # 8 token chunks (m on partitions)
RS = 16                    # raw rows per slab (two pooled rows)
TPS = (RS // S) * WP       # 64 tokens per slab

_CACHE: dict = {}


def _split_multi_waits(nc):
    """walrus in this container accepts at most ONE sync-wait per
    instruction; hoist extra waits onto inserted NoOps (same engine,
    right before the instruction -> identical semantics)."""
    import json

    d = json.loads(mybir.module_to_json_string(nc.m))
    n = 0
    for fn in d["functions"]:
        for bb in fn["blocks"]:
            out = []
            for inst in bb.get("instructions", []):
                si = inst.get("sync_info")
                waits = (si or {}).get("on_wait") or []
                if len(waits) > 1:
                    for w in waits[:-1]:
                        n += 1
                        out.append({
                            "debug": inst.get("debug", 0),
                            "engine": inst["engine"],
                            "ins": [], "outs": [],
                            "name": f"I-wsplit-{n}",
                            "opcode": "NoOp",
                            "sync_info": {"on_update": [], "on_wait": [w]},
                        })
                    si["on_wait"] = [waits[-1]]
                out.append(inst)
            bb["instructions"] = out
    nc.m = mybir.module_from_json_string(json.dumps(d))
    return n


def _build(split_waits=True):
    nc = bass.Bass(num_devices=N_CORES)

    xh = nc.declare_dram_parameter("xh", [C, HH, W], F32, isOutput=False)
    qw = nc.declare_dram_parameter("qw", [CCH, 128, KCH], BF16, isOutput=False)
    kw = nc.declare_dram_parameter("kw", [CCH, 128, KCH], BF16, isOutput=False)
    vw = nc.declare_dram_parameter("vw", [CCH, 128, C], BF16, isOutput=False)
    qb = nc.declare_dram_parameter("qb", [KCH], F32, isOutput=False)
    kb = nc.declare_dram_parameter("kb", [KCH], F32, isOutput=False)
    vb = nc.declare_dram_parameter("vb", [C], F32, isOutput=False)
    out = nc.declare_dram_parameter("out", [C, HH, W], F32, isOutput=True)

    with tile.TileContext(nc) as tc:
        with (
            tc.tile_pool(name="const", bufs=1) as constp,
            tc.tile_pool(name="cache", bufs=1) as cachep,
            tc.tile_pool(name="slab", bufs=3) as slabp,
            tc.tile_pool(name="wtree", bufs=2) as wtp,
            tc.tile_pool(name="attn", bufs=1) as attnp,
            tc.tile_pool(name="pqk", bufs=1, space="PSUM") as pqk,
            tc.tile_pool(name="pe", bufs=1, space="PSUM") as pep,
            tc.tile_pool(name="pacc", bufs=1, space="PSUM") as pacc,
            tc.tile_pool(name="dram", bufs=1, space="DRAM") as dram,
        ):
            # ---- constants / weights (scalar=ACT ring, executes first) ----
            qw_sb = [constp.tile([128, KCH], BF16, name=f"qw{k}") for k in range(CCH)]
            kw_sb = [constp.tile([128, KCH], BF16, name=f"kw{k}") for k in range(CCH)]
            vw_sb = [constp.tile([128, C], BF16, name=f"vw{k}") for k in range(CCH)]
            qb_sb = constp.tile([KCH, 1], F32, name="qb")
            kb_sb = constp.tile([KCH, 1], F32, name="kb")
            vb_col = [constp.tile([128, 1], F32, name=f"vb{k}")
                      for k in range(CCH)]
            for k in range(CCH):
                nc.scalar.dma_start(out=qw_sb[k][:], in_=qw[k])
                nc.scalar.dma_start(out=kw_sb[k][:], in_=kw[k])
                nc.scalar.dma_start(out=vw_sb[k][:], in_=vw[k])
            nc.scalar.dma_start(out=qb_sb[:], in_=qb[:])
            nc.scalar.dma_start(out=kb_sb[:], in_=kb[:])
            for k in range(CCH):
                nc.scalar.dma_start(out=vb_col[k][:],
                                    in_=vb[k * 128:(k + 1) * 128])
            ones128 = constp.tile([128, 128], F8, name="ones128")
            act_scr = constp.tile([1, 128], F32, name="act_scr")
            nc.vector.memset(ones128[:], 1.0)
            nc.vector.memset(act_scr[:], 0.0)
            # preload the ACT exp table before the attention needs it
            nc.scalar.activation(act_scr[:], act_scr[:],
                                 mybir.ActivationFunctionType.Exp)

            # pooled token sums (bf16), own half + gathered full set
            xf_bf = [constp.tile([128, NT], BF16, name=f"xfb{k}")
                     for k in range(CCH)]
            xf_full = [constp.tile([128, NTOK], BF16, name=f"xff{k}")
                       for k in range(CCH)]

            # persistent bf16 feature cache: 16 slabs of [128, 16, W]
            NSLAB = HH // RS
            cache = {}
            for t in range(NSLAB):
                for k in range(CCH):
                    cache[(k, t)] = cachep.tile([128, RS, W], BF16,
                                                name=f"cb{k}_{t}")

            # PSUM accumulators that persist across the whole attention
            psum_den = pacc.tile([128, NT], F32, tag="den", name="psum_den")
            psum_os = [pacc.tile([128, NT], F32, tag=f"pos{k}",
                                 name=f"psum_os{k}") for k in range(CCH)]

            # collective buffers (internal DRAM)
            NT2 = NT // 2
            cc_in = [dram.tile([C, NT2], BF16, name=f"cc_in{h}")
                     for h in range(2)]
            cc_out = [dram.tile([2, C, NT2], BF16, name=f"cc_out{h}")
                      for h in range(2)]

            def stage_collective(h):
                ts = slice(h * NT2, (h + 1) * NT2)
                for k in range(CCH):
                    nc.gpsimd.dma_start(
                        out=cc_in[h][k * 128:(k + 1) * 128, :],
                        in_=xf_bf[k][:, ts])

            def fire_collective(h):
                nc.gpsimd.collective_compute(
                    "AllGather", mybir.AluOpType.bypass,
                    replica_groups=[[0, 1], [2, 3], [4, 5], [6, 7]],
                    ins=[cc_in[h].opt()], outs=[cc_out[h].opt()],
                )

            def unpack_collective(h):
                for k in range(CCH):
                    src = cc_out[h][:, k * 128:(k + 1) * 128, :].rearrange(
                        "g c t -> c g t")
                    dstf = xf_full[k][:, h * NT:(h + 1) * NT].rearrange(
                        "c (g t) -> c g t", g=2)
                    nc.gpsimd.dma_start(out=dstf, in_=src)

            # ---- phase 1: stream slabs t-major on two DMA rings; cast to
            # bf16 cache (ACT); pool with an all-bf16 DVE tree READING THE
            # CACHE (2x DVE rate, slab freed by the cast alone).
            # KEY CONSTRAINT (measured): an active AllGather transfer window
            # completely starves the other DMA rings, so collective #1 is
            # timed to open its window right as the loads finish. ----
            idx = 0
            for t in range(NSLAB):
                rs = slice(t * RS, (t + 1) * RS)
                for k in range(CCH):
                    cs = slice(k * 128, (k + 1) * 128)
                    slab = slabp.tile([128, RS, W], F32, tag="slab",
                                      name="slab")
                    nc.sync.dma_start(out=slab[:], in_=xh[cs, rs, :])
                    nc.scalar.copy(cache[(k, t)].rearrange("c h w -> c (h w)"),
                                   slab.rearrange("c h w -> c (h w)"))
                    cb = cache[(k, t)]
                    l1 = wtp.tile([128, RS // 2, W], BF16, tag="l1", bufs=1,
                                  name="l1")
                    nc.vector.tensor_add(l1[:], cb[:, 0::2, :], cb[:, 1::2, :])
                    l2 = wtp.tile([128, RS // 4, W], BF16, tag="l2", bufs=1,
                                  name="l2")
                    nc.vector.tensor_add(l2[:], l1[:, 0::2, :], l1[:, 1::2, :])
                    l3 = wtp.tile([128, RS // 8, W], BF16, tag="l3", bufs=1,
                                  name="l3")
                    nc.vector.tensor_add(l3[:], l2[:, 0::2, :], l2[:, 1::2, :])
                    dst = xf_bf[k][:, t * TPS:(t + 1) * TPS]
                    with nc.allow_low_precision(
                            reason="pooled sums ~N(0,64); bf16 fine"):
                        nc.vector.reduce_sum(
                            dst, l3.rearrange("c i (wp r) -> c (i wp) r", r=S),
                            axis=mybir.AxisListType.X)
                    if idx == 14:
                        # cc#1 stores ride the sync DMA ring here (FIFO):
                        # their data lands as the last loads finish, so the
                        # collective window opens with no loads left to starve
                        for kk in range(CCH):
                            nc.sync.dma_start(
                                out=cc_in[0][kk * 128:(kk + 1) * 128, :],
                                in_=xf_bf[kk][:, 0:NT2])
                        fire_collective(0)
                    idx += 1

            # PE warm-up burst gated on the last slab's cast: trips the clock
            # ramp right before the real matmuls.  Writes den PSUM garbage;
            # den's first real matmul uses start=True so it is discarded.
            for _ in range(8):
                nc.tensor.matmul(psum_den[:KCH, 0:C], qw_sb[0][:],
                                 cache[(0, NSLAB - 1)][:, 0, :],
                                 start=True, stop=True)
            stage_collective(1)
            fire_collective(1)           # tokens 256:512
            unpack_collective(0)

            # ---- q projection (own tokens) ----
            q_sb = attnp.tile([KCH, NT], BF16, name="q_sb")
            psum_q = pqk.tile([KCH, NT], F32, tag="qk", name="psum_q")
            for k in range(CCH):
                nc.tensor.matmul(psum_q[:], qw_sb[k][:], xf_bf[k][:],
                                 start=(k == 0), stop=(k == CCH - 1))
            nc.vector.tensor_scalar_add(q_sb[:], psum_q[:], qb_sb[:])

            k_sb = attnp.tile([KCH, NTOK], BF16, name="k_sb")
            e_sb = [attnp.tile([128, NT], F8, name=f"e{j}") for j in range(NJ)]
            vt_sb = [attnp.tile([128, C], F8, name=f"vt{j}") for j in range(NJ)]

            def attn_half(h):
                # k projection for this half's 512 tokens
                ms = slice(h * NT, (h + 1) * NT)
                psum_k = pqk.tile([KCH, NT], F32, tag="qk", name="psum_k")
                for k in range(CCH):
                    nc.tensor.matmul(psum_k[:], kw_sb[k][:],
                                     xf_full[k][:, ms],
                                     start=(k == 0), stop=(k == CCH - 1))
                nc.vector.tensor_scalar_add(k_sb[:, ms], psum_k[:], kb_sb[:])
                # per 128-token m-chunk: v^T, energies+exp, den/os accumulate
                for j in range(h * NJ // 2, (h + 1) * NJ // 2):
                    js = slice(j * 128, (j + 1) * 128)
                    psum_vt = pep.tile([128, C], F32, tag="pvt", name="psum_vt")
                    for k in range(CCH):
                        nc.tensor.matmul(psum_vt[:], xf_full[k][:, js],
                                         vw_sb[k][:],
                                         start=(k == 0), stop=(k == CCH - 1))
                    nc.vector.tensor_copy(vt_sb[j][:], psum_vt[:])

                    psum_e = pep.tile([128, NT], F32, tag="pe", bufs=2,
                                      name="psum_e")
                    nc.tensor.matmul(psum_e[:], k_sb[:, js], q_sb[:],
                                     start=True, stop=True)
                    nc.scalar.activation(e_sb[j][:], psum_e[:],
                                         mybir.ActivationFunctionType.Exp)
                    nc.tensor.matmul(psum_den[:], ones128[:], e_sb[j][:],
                                     start=(j == 0), stop=(j == NJ - 1))
                    for k in range(CCH):
                        nc.tensor.matmul(psum_os[k][:],
                                         vt_sb[j][:, k * 128:(k + 1) * 128],
                                         e_sb[j][:],
                                         start=(j == 0), stop=(j == NJ - 1))

            attn_half(0)
            unpack_collective(1)
            attn_half(1)

            # ---- normalize in column halves so early stores start sooner ----
            recip = attnp.tile([128, NT], F32, name="recip")
            os_sb = [attnp.tile([128, NT], BF16, name=f"os{k}")
                     for k in range(CCH)]
            NT4 = NT // 4
            for h in range(4):
                hs = slice(h * NT4, (h + 1) * NT4)
                nc.vector.reciprocal(recip[:, hs], psum_den[:, hs])
                for k in range(CCH):
                    nc.vector.tensor_mul(os_sb[k][:, hs], psum_os[k][:, hs],
                                         recip[:, hs])
                    nc.vector.tensor_scalar_add(os_sb[k][:, hs],
                                                os_sb[k][:, hs], vb_col[k][:])

            # ---- tail: add upsampled attention to the bf16 cache, store;
            # adds alternate DVE/gpsimd, stores alternate scalar/sync ----
            idx = 0
            for t in range(NSLAB):
                rs = slice(t * RS, (t + 1) * RS)
                for k in range(CCH):
                    cs = slice(k * 128, (k + 1) * 128)
                    stg = slabp.tile([128, RS, W], F32, tag="slab", name="stg")
                    src = bass.AP(os_sb[k].tensor,
                                  os_sb[k].offset + t * WP,
                                  [list(os_sb[k].ap[0]),
                                   [0, RS], [1, WP], [0, S]])
                    nc.vector.tensor_add(
                        stg.rearrange("c h (wp wr) -> c h wp wr", wr=S),
                        cache[(k, t)].rearrange("c h (wp wr) -> c h wp wr",
                                                wr=S),
                        src)
                    eng = nc.scalar if idx % 2 == 0 else nc.sync
                    eng.dma_start(out=out[cs, rs, :], in_=stg[:])
                    idx += 1

    if split_waits:
        _split_multi_waits(nc)
    return nc


def _get_nc():
    if "nc" not in _CACHE:
        _CACHE["nc"] = _build()
    return _CACHE["nc"]


def kernel(features, q_w, q_b, k_w, k_b, v_w, v_b):
    nc = _get_nc()
    inv = 1.0 / (S * S)
    scale = float(KCH) ** -0.5
    qw_eff = np.ascontiguousarray(
        (q_w.T * (scale * inv)).astype(ml_dtypes.bfloat16).reshape(CCH, 128, KCH))
    qb_eff = np.ascontiguousarray((q_b * scale).astype(np.float32))
    kw_eff = np.ascontiguousarray(
        (k_w.T * inv).astype(ml_dtypes.bfloat16).reshape(CCH, 128, KCH))
    kb_eff = np.ascontiguousarray(k_b.astype(np.float32))
    vw_eff = np.ascontiguousarray(
        (v_w.T * inv).astype(ml_dtypes.bfloat16).reshape(CCH, 128, C))
    vb_eff = np.ascontiguousarray(v_b.astype(np.float32))

    features = np.asarray(features, dtype=np.float32)
    in_maps = []
    for i in range(N_CORES):
        b, half = i // 2, i % 2
        in_maps.append({
            "xh": np.ascontiguousarray(
                features[b, :, half * HH:(half + 1) * HH, :]),
            "qw": qw_eff, "kw": kw_eff, "vw": vw_eff,
            "qb": qb_eff, "kb": kb_eff, "vb": vb_eff,
        })

    res = run_bass_kernel_spmd(nc, in_maps, list(range(N_CORES)))
    out = np.empty((B, C, H, W), dtype=np.float32)
    for i in range(N_CORES):
        b, half = i // 2, i % 2
        out[b, :, half * HH:(half + 1) * HH, :] = res.results[i]["out"]
    return out
